# revision 16
# baseline (speedup 1.0000x reference)
"""FFT-block kernel for Trainium2 (8 NeuronCores, batch-data-parallel).

Computation (per sample):
  y0  = mean(x, (H, W))                      [C]
  h   = relu(y0 @ W1c.T + b1)                [C/6]
  y   = sigmoid(h @ W2c.T + b2)              [C]
  s1  = relu(y @ Ws1.T + bs1)                [CF]
  s2  = relu(y @ Ws2.T + bs2)                [CF]
  yf  = rfft(y); amp=|yf|*s1; pha=angle(yf)*s2
  rec = amp*(cos(pha) + i sin(pha)); xr = irfft(rec, C)
  out = (xr * y)[:, None, None]

Strategy: batch dim (16) sharded 2-per-core. The 400MB stream of x feeds a
free-axis reduction (DMA-bound, split between the DVE reduce and the ACT
accum_out path so neither engine paces the stream). The tiny MLP tail runs
channel-major ([chan, batch]) through the squeeze-excite matmuls, then
flips batch-major ([batch, freq]) via PE transposes for the rfft
projections, elementwise trig, irfft (all matmuls against host-precomputed
DFT bases) and a contiguous output store.
angle() uses atan2(y,x) = 2*atan(y/(|z|+x)) with the atan argument folded
into [0,1] (atan(1/x) identity); sin/cos use an exact mod-2pi range
reduction (fp32 round-to-int magic constant). The DC and Nyquist bins
(Im==0 analytically) are special-cased via sign(Re).
HW activation-table ranges: Arctan needs |x| <= pi/2, Sin |x| <= pi.
"""

import numpy as np
from contextlib import ExitStack

import concourse.bass as bass
import concourse.bacc as bacc
import concourse.tile as tile
from concourse import mybir
from concourse.bass_utils import run_bass_kernel_spmd

B, C, H, W = 16, 384, 128, 128
NCORES = 8
BPC = B // NCORES            # 2 samples per core
CH = C // 6                  # 64
CF = C // 2 + 1              # 193 rfft bins
HW = H * W                   # 16384
FP32 = mybir.dt.float32
AF = mybir.ActivationFunctionType
AX = mybir.AxisListType
OP = mybir.AluOpType

F_CHUNK = 4096               # free-dim chunk of the x stream
N_CHUNK = HW // F_CHUNK      # 4
STREAM_BUFS = 8

KC = [0, 128, 256]           # channel chunks (3 x 128)
FC = [(0, 128), (128, 65)]   # freq-bin chunks (128 + 65)
NYQ = 192                    # Nyquist bin index
EPS = 1e-30
MAGIC = 12582912.0           # 1.5 * 2**23: x+MAGIC-MAGIC == round(x) in fp32


def _build():
    nc = bacc.Bacc(
        "TRN2",
        target_bir_lowering=False,
        debug=False,
        enable_asserts=False,
        num_devices=NCORES,
    )

    xs = nc.dram_tensor("xs", [BPC, C, H, W], FP32, kind="ExternalInput")
    w1t = nc.dram_tensor("w1t", [C, CH], FP32, kind="ExternalInput")
    b1c = nc.dram_tensor("b1c", [CH, 1], FP32, kind="ExternalInput")
    w2t = nc.dram_tensor("w2t", [CH, C], FP32, kind="ExternalInput")
    b2c = nc.dram_tensor("b2c", [C, 1], FP32, kind="ExternalInput")
    ws1t = nc.dram_tensor("ws1t", [C, CF], FP32, kind="ExternalInput")
    ws2t = nc.dram_tensor("ws2t", [C, CF], FP32, kind="ExternalInput")
    bs1r = nc.dram_tensor("bs1r", [BPC, CF], FP32, kind="ExternalInput")
    bs2r = nc.dram_tensor("bs2r", [BPC, CF], FP32, kind="ExternalInput")
    cmat = nc.dram_tensor("cmat", [C, CF], FP32, kind="ExternalInput")
    smat = nc.dram_tensor("smat", [C, CF], FP32, kind="ExternalInput")
    icrm = nc.dram_tensor("icrm", [CF, C], FP32, kind="ExternalInput")
    icim = nc.dram_tensor("icim", [CF, C], FP32, kind="ExternalInput")
    idmat = nc.dram_tensor("idmat", [128, 128], FP32, kind="ExternalInput")
    outp = nc.dram_tensor("out", [BPC, C, 1, 1], FP32, kind="ExternalOutput")

    with tile.TileContext(nc) as tc, ExitStack() as ctx:
        persist = ctx.enter_context(tc.tile_pool(name="persist", bufs=1))
        stream = ctx.enter_context(tc.tile_pool(name="stream", bufs=STREAM_BUFS))
        # PSUM budget is 8 banks; every tile here is <= 1 bank.
        ps_mm = ctx.enter_context(
            tc.tile_pool(name="ps_mm", bufs=2, space=bass.MemorySpace.PSUM)
        )
        ps_pg = ctx.enter_context(
            tc.tile_pool(name="ps_pg", bufs=2, space=bass.MemorySpace.PSUM)
        )
        ps_tr = ctx.enter_context(
            tc.tile_pool(name="ps_tr", bufs=2, space=bass.MemorySpace.PSUM)
        )
        ps_big = ctx.enter_context(
            tc.tile_pool(name="ps_big", bufs=1, space=bass.MemorySpace.PSUM)
        )

        def load_const(tag, dram_ap, shape):
            t = persist.tile(shape, FP32, tag=tag, name=tag)
            nc.gpsimd.dma_start(out=t, in_=dram_ap)
            return t

        # ---- small constants (loaded once, overlap with the x stream) ----
        w1t_sb = [load_const(f"w1t{k}", w1t[s : s + 128, :], [128, CH]) for k, s in enumerate(KC)]
        w2t_sb = load_const("w2t", w2t[:, :], [CH, C])
        b1_sb = load_const("b1", b1c[:, :], [CH, 1])
        b2_sb = [load_const(f"b2_{m}", b2c[s : s + 128, :], [128, 1]) for m, s in enumerate(KC)]
        proj_mats = {}
        for nm, dr in (("ws1t", ws1t), ("ws2t", ws2t), ("cmat", cmat), ("smat", smat)):
            proj_mats[nm] = [
                load_const(f"{nm}{k}", dr[s : s + 128, :], [128, CF]) for k, s in enumerate(KC)
            ]
        bs1_sb = load_const("bs1r", bs1r[:, :], [BPC, CF])
        bs2_sb = load_const("bs2r", bs2r[:, :], [BPC, CF])
        icr_sb = [load_const(f"icr{j}", icrm[s : s + l, :], [l, C]) for j, (s, l) in enumerate(FC)]
        ici_sb = [load_const(f"ici{j}", icim[s : s + l, :], [l, C]) for j, (s, l) in enumerate(FC)]
        id_sb = load_const("idmat", idmat[:, :], [128, 128])

        # ---- phase 1: stream x, per-(b,c)-row sums over the spatial axis ----
        # rows of [BPC*C, HW]; row-tile t holds channels of (b = t//3, kchunk = t%3).
        # Chunks alternate between DVE (reduce) and ACT (Identity + accum_out
        # row-sum side output) so neither engine alone paces the DMA stream.
        xrows = xs.rearrange("b c h w -> (b c) (h w)")
        dummy = persist.tile([128, F_CHUNK], mybir.dt.bfloat16, tag="dummy", name="dummy")
        yt = [persist.tile([128, BPC], FP32, tag=f"yt{k}", name=f"yt{k}") for k in range(3)]
        for t in range(BPC * 3):
            b, k = divmod(t, 3)
            # one partial tile per chunk: shared tiles would create false
            # WAW deps in Tile's tracker and serialize the two reduce engines
            parts = [
                persist.tile([128, 1], FP32, tag=f"part{t}_{j}", name=f"part{t}_{j}")
                for j in range(N_CHUNK)
            ]
            for j in range(N_CHUNK):
                chk = stream.tile([128, F_CHUNK], FP32, tag="stream", name=f"chk{t}_{j}")
                nc.sync.dma_start(
                    out=chk,
                    in_=xrows[t * 128 : (t + 1) * 128, j * F_CHUNK : (j + 1) * F_CHUNK],
                )
                if j % 2 == 0:
                    nc.vector.reduce_sum(out=parts[j], in_=chk, axis=AX.X)
                else:
                    nc.scalar.activation(
                        out=dummy[:, :], in_=chk, func=AF.Identity,
                        accum_out=parts[j],
                    )
            a01 = persist.tile([128, 1], FP32, tag=f"a01_{t}", name=f"a01_{t}")
            nc.vector.tensor_add(out=a01, in0=parts[0], in1=parts[1])
            a23 = persist.tile([128, 1], FP32, tag=f"a23_{t}", name=f"a23_{t}")
            nc.vector.tensor_add(out=a23, in0=parts[2], in1=parts[3])
            nc.vector.tensor_add(out=yt[k][:, b : b + 1], in0=a01, in1=a23)
        # yt holds raw sums; the 1/HW mean scale is folded into w1t host-side.

        # ---- phase 2a: squeeze-excite MLP, channel-major [chan, batch] ----
        ph = ps_mm.tile([CH, BPC], FP32, tag="mm", name="ph")
        for k in range(3):
            nc.tensor.matmul(ph, lhsT=w1t_sb[k], rhs=yt[k], start=(k == 0), stop=(k == 2))
        h_sb = persist.tile([CH, BPC], FP32, tag="h", name="h_sb")
        nc.scalar.activation(out=h_sb, in_=ph, func=AF.Relu, bias=b1_sb)

        y_sb = []
        for m, s in enumerate(KC):
            py = ps_mm.tile([128, BPC], FP32, tag="mm", name=f"py{m}")
            nc.tensor.matmul(py, lhsT=w2t_sb[:, s : s + 128], rhs=h_sb, start=True, stop=True)
            yb = persist.tile([128, BPC], FP32, tag=f"y{m}", name=f"y{m}")
            nc.scalar.activation(out=yb, in_=py, func=AF.Sigmoid, bias=b2_sb[m])
            y_sb.append(yb)

        # y batch-major [BPC, C] via PE transposes (for the final xr*y and DMA)
        pyt = ps_big.tile([BPC, C], FP32, tag="pyt", name="pyt")
        for m, s in enumerate(KC):
            nc.tensor.transpose(pyt[:, s : s + 128], y_sb[m], id_sb)
        yb2 = persist.tile([BPC, C], FP32, tag="yb2", name="yb2")
        nc.vector.tensor_copy(out=yb2, in_=pyt)

        # ---- phase 2b: rfft projections, batch-major [BPC, CF] ----
        def proj_bm(mats, nm):
            pg = ps_pg.tile([BPC, CF], FP32, tag="pg", name=f"pg_{nm}")
            for k in range(3):
                nc.tensor.matmul(
                    pg, lhsT=y_sb[k], rhs=mats[k][:, :], start=(k == 0), stop=(k == 2)
                )
            ob = persist.tile([BPC, CF], FP32, tag=f"bm_{nm}", name=f"bm_{nm}")
            return pg, ob

        pg1, s1b = proj_bm(proj_mats["ws1t"], "s1")
        nc.vector.tensor_add(out=s1b, in0=pg1, in1=bs1_sb)
        nc.vector.tensor_scalar_max(out=s1b, in0=s1b, scalar1=0.0)
        pg2, s2b = proj_bm(proj_mats["ws2t"], "s2")
        nc.vector.tensor_add(out=s2b, in0=pg2, in1=bs2_sb)
        nc.vector.tensor_scalar_max(out=s2b, in0=s2b, scalar1=0.0)
        pgr, reb = proj_bm(proj_mats["cmat"], "re")
        nc.vector.tensor_copy(out=reb, in_=pgr)
        pgi, imb = proj_bm(proj_mats["smat"], "im")
        nc.vector.tensor_copy(out=imb, in_=pgi)

        # ---- phase 2c: rec = amp * exp(i*pha), batch-major, one pass ----
        def T(tg):
            return persist.tile([BPC, CF], FP32, tag=tg, name=tg)

        r2 = T("r2")
        nc.vector.tensor_mul(out=r2, in0=reb, in1=reb)
        i2 = T("i2")
        nc.vector.tensor_mul(out=i2, in0=imb, in1=imb)
        nc.vector.tensor_add(out=r2, in0=r2, in1=i2)
        amp0 = T("amp0")
        nc.scalar.activation(out=amp0, in_=r2, func=AF.Sqrt)
        den = T("den")  # |z| + Re + eps, then reciprocal
        nc.vector.scalar_tensor_tensor(
            out=den, in0=amp0, scalar=EPS, in1=reb, op0=OP.add, op1=OP.add
        )
        nc.vector.reciprocal(out=den, in_=den)
        tq = T("tq")  # t = Im / (|z| + Re) = tan(angle/2)
        nc.vector.tensor_mul(out=tq, in0=imb, in1=den)
        ab = T("ab")  # |t| = max(-t, t), kept strictly positive
        nc.vector.scalar_tensor_tensor(
            out=ab, in0=tq, scalar=-1.0, in1=tq, op0=OP.mult, op1=OP.max
        )
        nc.vector.tensor_scalar_add(out=ab, in0=ab, scalar1=EPS)
        inv = T("inv")
        nc.vector.reciprocal(out=inv, in_=ab)
        u = T("u")
        nc.vector.tensor_tensor(out=u, in0=ab, in1=inv, op=OP.min)
        a = T("a")  # atan(u) in [0, pi/4]
        nc.scalar.activation(out=a, in_=u, func=AF.Arctan)
        w = T("w")  # pi/2 - 2a, applied where |t| > 1
        nc.vector.tensor_scalar(
            out=w, in0=a, scalar1=-2.0, scalar2=float(np.pi / 2),
            op0=OP.mult, op1=OP.add,
        )
        fgt = T("fgt")
        nc.vector.tensor_scalar(
            out=fgt, in0=ab, scalar1=1.0, scalar2=None, op0=OP.is_gt
        )
        nc.vector.tensor_mul(out=w, in0=w, in1=fgt)
        nc.vector.tensor_add(out=a, in0=a, in1=w)  # |angle|/2
        f0 = T("f0")  # sign(t) as 2*(t>0)-1, folded: at = 2*a*f0 - a
        nc.vector.tensor_scalar(
            out=f0, in0=tq, scalar1=0.0, scalar2=None, op0=OP.is_gt
        )
        af = T("af")
        nc.vector.tensor_mul(out=af, in0=a, in1=f0)
        at = T("at")  # angle/2
        nc.vector.scalar_tensor_tensor(
            out=at, in0=af, scalar=2.0, in1=a, op0=OP.mult, op1=OP.subtract
        )
        # DC / Nyquist bins: Im==0 analytically, the half-angle quotient is
        # noise-driven. angle is 0 (Re>0) or pi (Re<0): at = pi/2*(1-(Re>0)).
        for p in (0, NYQ):
            fp = persist.tile([BPC, 1], FP32, tag=f"fp{p}", name=f"fp{p}")
            nc.vector.tensor_scalar(
                out=fp, in0=reb[:, p : p + 1], scalar1=0.0, scalar2=None, op0=OP.is_gt
            )
            nc.vector.tensor_scalar(
                out=at[:, p : p + 1], in0=fp,
                scalar1=float(-np.pi / 2), scalar2=float(np.pi / 2),
                op0=OP.mult, op1=OP.add,
            )
        q = T("q")  # q = (angle/2)*s2; pha = 2q
        nc.vector.tensor_mul(out=q, in0=at, in1=s2b)
        r = T("r")  # pha / 2pi
        nc.vector.tensor_scalar_mul(out=r, in0=q, scalar1=float(1.0 / np.pi))
        rc = T("rc")  # (pha + pi/2) / 2pi
        nc.vector.tensor_scalar_add(out=rc, in0=r, scalar1=0.25)
        n1 = T("n1")
        nc.vector.tensor_scalar(
            out=n1, in0=r, scalar1=MAGIC, scalar2=MAGIC, op0=OP.add, op1=OP.subtract
        )
        nc.vector.tensor_sub(out=n1, in0=r, in1=n1)  # frac(r) in [-.5, .5]
        sn = T("sn")
        nc.scalar.activation(out=sn, in_=n1, func=AF.Sin, scale=float(2 * np.pi))
        n2 = T("n2")
        nc.vector.tensor_scalar(
            out=n2, in0=rc, scalar1=MAGIC, scalar2=MAGIC, op0=OP.add, op1=OP.subtract
        )
        nc.vector.tensor_sub(out=n2, in0=rc, in1=n2)
        cs = T("cs")
        nc.scalar.activation(out=cs, in_=n2, func=AF.Sin, scale=float(2 * np.pi))
        amp = T("amp")
        nc.vector.tensor_mul(out=amp, in0=amp0, in1=s1b)
        rr = T("rr")
        nc.vector.tensor_mul(out=rr, in0=amp, in1=cs)
        ri = T("ri")
        nc.vector.tensor_mul(out=ri, in0=amp, in1=sn)

        # ---- phase 2d: rec back to freq-major chunks via PE transposes ----
        recT = []
        for src, nm in ((rr, "rr"), (ri, "ri")):
            chunks = []
            for j, (s, l) in enumerate(FC):
                pt = ps_tr.tile([l, BPC], FP32, tag="tr", name=f"pt_{nm}{j}")
                nc.tensor.transpose(pt, src[:, s : s + l], id_sb[:BPC, :BPC])
                ct = persist.tile([l, BPC], FP32, tag=f"{nm}T{j}", name=f"{nm}T{j}")
                nc.vector.tensor_copy(out=ct, in_=pt)
                chunks.append(ct)
            recT.append(chunks)
        rrT, riT = recT

        # ---- phase 2e: irfft + final scale, batch-major; contiguous store ----
        pxr = ps_big.tile([BPC, C], FP32, tag="pxr", name="pxr")
        steps = [
            (rrT[0], icr_sb[0]), (rrT[1], icr_sb[1]),
            (riT[0], ici_sb[0]), (riT[1], ici_sb[1]),
        ]
        for idx, (lt, rs) in enumerate(steps):
            nc.tensor.matmul(
                pxr, lhsT=lt, rhs=rs[:, :], start=(idx == 0), stop=(idx == len(steps) - 1)
            )
        out_sb = persist.tile([BPC, C], FP32, tag="out_sb", name="out_sb")
        nc.vector.tensor_mul(out=out_sb, in0=pxr, in1=yb2)
        base = outp.ap()
        dst = bass.AP(tensor=base.tensor, offset=0, ap=[[C, BPC], [1, C]])
        nc.sync.dma_start(out=dst, in_=out_sb)

    nc.compile()
    return nc


_CACHE = {}


def _get_nc():
    if "nc" not in _CACHE:
        _CACHE["nc"] = _build()
    return _CACHE["nc"]


def _host_prep(inputs):
    f32 = np.float32
    W1 = np.asarray(inputs["W1"], f32)
    W2 = np.asarray(inputs["W2"], f32)
    Ws1 = np.asarray(inputs["Ws1"], f32)
    Ws2 = np.asarray(inputs["Ws2"], f32)
    # center taps of the 3x3 convs; fold the 1/HW mean scale into W1
    w1t = np.ascontiguousarray(W1[:, :, 1, 1].T.astype(np.float64) / HW).astype(f32)
    w2t = np.ascontiguousarray(W2[:, :, 1, 1].T)
    ws1t = np.ascontiguousarray(Ws1.T)
    ws2t = np.ascontiguousarray(Ws2.T)

    i = np.arange(C, dtype=np.float64)[:, None]
    k = np.arange(CF, dtype=np.float64)[None, :]
    ang = 2.0 * np.pi * i * k / C
    cmat = np.cos(ang).astype(f32)
    smat = (-np.sin(ang)).astype(f32)

    kk = np.arange(CF, dtype=np.float64)[:, None]
    n = np.arange(C, dtype=np.float64)[None, :]
    ang2 = 2.0 * np.pi * kk * n / C
    alpha = np.full((CF, 1), 2.0)
    alpha[0, 0] = 1.0
    alpha[NYQ, 0] = 1.0
    icrm = (alpha * np.cos(ang2) / C).astype(f32)
    icim = (-alpha * np.sin(ang2) / C).astype(f32)

    return dict(
        w1t=w1t,
        b1c=np.ascontiguousarray(np.asarray(inputs["b1"], f32).reshape(CH, 1)),
        w2t=w2t,
        b2c=np.ascontiguousarray(np.asarray(inputs["b2"], f32).reshape(C, 1)),
        ws1t=ws1t,
        ws2t=ws2t,
        bs1r=np.ascontiguousarray(np.tile(np.asarray(inputs["bs1"], f32), (BPC, 1))),
        bs2r=np.ascontiguousarray(np.tile(np.asarray(inputs["bs2"], f32), (BPC, 1))),
        cmat=cmat,
        smat=smat,
        icrm=icrm,
        icim=icim,
        idmat=np.eye(128, dtype=f32),
    )


def kernel(**inputs):
    x = np.asarray(inputs["x"], np.float32)
    base = _host_prep(inputs)
    nc = _get_nc()
    in_maps = [
        dict(base, xs=np.ascontiguousarray(x[i * BPC : (i + 1) * BPC]))
        for i in range(NCORES)
    ]
    res = run_bass_kernel_spmd(nc, in_maps, list(range(NCORES))).results
    return np.concatenate([res[i]["out"] for i in range(NCORES)], axis=0)


# revision 17
# speedup vs baseline: 1.0976x; 1.0976x over previous
"""FFT-block kernel for Trainium2 (8 NeuronCores, batch-data-parallel).

Computation (per sample):
  y0  = mean(x, (H, W))                      [C]
  h   = relu(y0 @ W1c.T + b1)                [C/6]
  y   = sigmoid(h @ W2c.T + b2)              [C]
  s1  = relu(y @ Ws1.T + bs1)                [CF]
  s2  = relu(y @ Ws2.T + bs2)                [CF]
  yf  = rfft(y); amp=|yf|*s1; pha=angle(yf)*s2
  rec = amp*(cos(pha) + i sin(pha)); xr = irfft(rec, C)
  out = (xr * y)[:, None, None]

Strategy: batch dim (16) sharded 2-per-core. The 400MB stream of x feeds a
free-axis reduction (DMA-bound; chunks alternate between the DVE reduce and
the ACT accum_out path so neither engine paces the stream). All constants
are packed host-side into one [128, TOTW] buffer loaded by a single DMA
(many small weight DMAs measurably degrade early stream bandwidth).
The tiny MLP/FFT tail runs freq-major ([chan/freq, batch]): rfft/irfft are
matmuls against host-precomputed DFT bases (moving free dim stays at
BPC=2 because fp32 matmul pays ~4 cycles per moving column); the final
result is PE-transposed to [batch, C] for one contiguous store.
angle() uses atan2(y,x) = 2*atan(y/(|z|+x)) with the atan argument folded
into [0,1] (atan(1/x) identity); sin/cos use an exact mod-2pi range
reduction (fp32 round-to-int magic constant). The DC and Nyquist bins
(Im==0 analytically) are special-cased via sign(Re). Non-transcendental
pieces run on DVE (tensor_scalar / scalar_tensor_tensor fusions) to
minimize ACT table swaps (Arctan needs |x|<=pi/2, Sin |x|<=pi).
"""

import numpy as np
from contextlib import ExitStack

import concourse.bass as bass
import concourse.bacc as bacc
import concourse.tile as tile
from concourse import mybir
from concourse.bass_utils import run_bass_kernel_spmd

B, C, H, W = 16, 384, 128, 128
NCORES = 8
BPC = B // NCORES            # 2 samples per core
CH = C // 6                  # 64
CF = C // 2 + 1              # 193 rfft bins
HW = H * W                   # 16384
FP32 = mybir.dt.float32
AF = mybir.ActivationFunctionType
AX = mybir.AxisListType
OP = mybir.AluOpType

F_CHUNK = 4096               # free-dim chunk of the x stream
N_CHUNK = HW // F_CHUNK      # 4
STREAM_BUFS = 8

KC = [0, 128, 256]           # channel chunks (3 x 128)
FC = [(0, 128), (128, 65)]   # freq-bin chunks (128 + 65)
NYQ = 192                    # Nyquist bin index
EPS = 1e-30
MAGIC = 12582912.0           # 1.5 * 2**23: x+MAGIC-MAGIC == round(x) in fp32

# ---- packed-constant column layout (shared by host prep and the build) ----
_OFF = {}
_tot = 0


def _alloc_cols(name, ncols):
    global _tot
    _OFF[name] = _tot
    _tot += ncols


for _k in range(3):
    _alloc_cols(f"w1t{_k}", CH)
_alloc_cols("w2t", C)
_alloc_cols("b1", 1)
for _m in range(3):
    _alloc_cols(f"b2_{_m}", 1)
for _nm in ("ws1t", "ws2t", "cmat", "smat"):
    for _k in range(3):
        _alloc_cols(f"{_nm}{_k}", CF)
for _j in range(2):
    _alloc_cols(f"bs1_{_j}", 1)
    _alloc_cols(f"bs2_{_j}", 1)
for _j in range(2):
    _alloc_cols(f"icr{_j}", C)
    _alloc_cols(f"ici{_j}", C)
_alloc_cols("idmat", 128)
TOTW = _tot


def _build():
    nc = bacc.Bacc(
        "TRN2",
        target_bir_lowering=False,
        debug=False,
        enable_asserts=False,
        num_devices=NCORES,
    )

    xs = nc.dram_tensor("xs", [BPC, C, H, W], FP32, kind="ExternalInput")
    wpk = nc.dram_tensor("wpk", [128, TOTW], FP32, kind="ExternalInput")
    outp = nc.dram_tensor("out", [BPC, C, 1, 1], FP32, kind="ExternalOutput")

    with tile.TileContext(nc) as tc, ExitStack() as ctx:
        persist = ctx.enter_context(tc.tile_pool(name="persist", bufs=1))
        stream = ctx.enter_context(tc.tile_pool(name="stream", bufs=STREAM_BUFS))
        # PSUM budget is 8 banks; every tile here is <= 1 bank.
        ps_mm = ctx.enter_context(
            tc.tile_pool(name="ps_mm", bufs=6, space=bass.MemorySpace.PSUM)
        )
        ps_fin = ctx.enter_context(
            tc.tile_pool(name="ps_fin", bufs=1, space=bass.MemorySpace.PSUM)
        )

        # ---- all constants in one DMA (issued on the ACT queue so the sync
        # queue runs the x stream exclusively) ----
        wp = persist.tile([128, TOTW], FP32, tag="wp", name="wp")
        nc.scalar.dma_start(out=wp, in_=wpk[:, :])

        def cslice(name, rows, ncols):
            o = _OFF[name]
            return wp[:rows, o : o + ncols]

        w1t_sb = [cslice(f"w1t{k}", 128, CH) for k in range(3)]
        w2t_sb = cslice("w2t", CH, C)
        b1_sb = cslice("b1", CH, 1)
        b2_sb = [cslice(f"b2_{m}", 128, 1) for m in range(3)]
        mats = {
            nm: [cslice(f"{nm}{k}", 128, CF) for k in range(3)]
            for nm in ("ws1t", "ws2t", "cmat", "smat")
        }
        bs1_sb = [cslice(f"bs1_{j}", l, 1) for j, (s, l) in enumerate(FC)]
        bs2_sb = [cslice(f"bs2_{j}", l, 1) for j, (s, l) in enumerate(FC)]
        icr_sb = [cslice(f"icr{j}", l, C) for j, (s, l) in enumerate(FC)]
        ici_sb = [cslice(f"ici{j}", l, C) for j, (s, l) in enumerate(FC)]
        id_sb = cslice("idmat", 128, 128)

        # ---- phase 1: stream x, per-(b,c)-row sums over the spatial axis ----
        # rows of [BPC*C, HW]; row-tile t holds channels of (b = t//3, kchunk = t%3).
        xrows = xs.rearrange("b c h w -> (b c) (h w)")
        dummy = persist.tile([128, F_CHUNK], mybir.dt.bfloat16, tag="dummy", name="dummy")
        yt = [persist.tile([128, BPC], FP32, tag=f"yt{k}", name=f"yt{k}") for k in range(3)]
        for t in range(BPC * 3):
            b, k = divmod(t, 3)
            # one partial tile per chunk: shared tiles would create false
            # WAW deps in Tile's tracker and serialize the two reduce engines
            parts = [
                persist.tile([128, 1], FP32, tag=f"part{t}_{j}", name=f"part{t}_{j}")
                for j in range(N_CHUNK)
            ]
            for j in range(N_CHUNK):
                chk = stream.tile([128, F_CHUNK], FP32, tag="stream", name=f"chk{t}_{j}")
                nc.sync.dma_start(
                    out=chk,
                    in_=xrows[t * 128 : (t + 1) * 128, j * F_CHUNK : (j + 1) * F_CHUNK],
                )
                if j % 2 == 0:
                    nc.vector.reduce_sum(out=parts[j], in_=chk, axis=AX.X)
                else:
                    nc.scalar.activation(
                        out=dummy[:, :], in_=chk, func=AF.Identity,
                        accum_out=parts[j],
                    )
            a01 = persist.tile([128, 1], FP32, tag=f"a01_{t}", name=f"a01_{t}")
            nc.vector.tensor_add(out=a01, in0=parts[0], in1=parts[1])
            a23 = persist.tile([128, 1], FP32, tag=f"a23_{t}", name=f"a23_{t}")
            nc.vector.tensor_add(out=a23, in0=parts[2], in1=parts[3])
            nc.vector.tensor_add(out=yt[k][:, b : b + 1], in0=a01, in1=a23)
        # yt holds raw sums; the 1/HW mean scale is folded into w1t host-side.

        # ---- phase 2: squeeze-excite MLP, freq-major [chan/freq, batch] ----
        ph = ps_mm.tile([CH, BPC], FP32, tag="mm", name="ph")
        for k in range(3):
            nc.tensor.matmul(ph, lhsT=w1t_sb[k], rhs=yt[k], start=(k == 0), stop=(k == 2))
        h_sb = persist.tile([CH, BPC], FP32, tag="h", name="h_sb")
        nc.scalar.activation(out=h_sb, in_=ph, func=AF.Relu, bias=b1_sb)

        y_sb = []
        for m, s in enumerate(KC):
            py = ps_mm.tile([128, BPC], FP32, tag="mm", name=f"py{m}")
            nc.tensor.matmul(py, lhsT=w2t_sb[:, s : s + 128], rhs=h_sb, start=True, stop=True)
            yb = persist.tile([128, BPC], FP32, tag=f"y{m}", name=f"y{m}")
            nc.scalar.activation(out=yb, in_=py, func=AF.Sigmoid, bias=b2_sb[m])
            y_sb.append(yb)

        # projections of y through [C, CF] matrices -> [CF, BPC] in 2 chunks.
        # s1/s2 post-ops (bias + relu) run on DVE to avoid ACT table swaps.
        def proj(mset, nm):
            outs = []
            for j, (s, l) in enumerate(FC):
                pt = ps_mm.tile([l, BPC], FP32, tag="mm", name=f"p{nm}{j}")
                for k in range(3):
                    nc.tensor.matmul(
                        pt, lhsT=mset[k][:, s : s + l], rhs=y_sb[k],
                        start=(k == 0), stop=(k == 2),
                    )
                outs.append(pt)
            return outs

        def sbuf2(nm):
            return [
                persist.tile([l, BPC], FP32, tag=f"{nm}{j}", name=f"{nm}{j}")
                for j, (s, l) in enumerate(FC)
            ]

        s1 = sbuf2("s1")
        for j, pt in enumerate(proj(mats["ws1t"], "s1")):
            l = FC[j][1]
            nc.vector.tensor_add(out=s1[j], in0=pt, in1=bs1_sb[j].to_broadcast([l, BPC]))
            nc.vector.tensor_scalar_max(out=s1[j], in0=s1[j], scalar1=0.0)
        s2 = sbuf2("s2")
        for j, pt in enumerate(proj(mats["ws2t"], "s2")):
            l = FC[j][1]
            nc.vector.tensor_add(out=s2[j], in0=pt, in1=bs2_sb[j].to_broadcast([l, BPC]))
            nc.vector.tensor_scalar_max(out=s2[j], in0=s2[j], scalar1=0.0)
        re = sbuf2("re")
        for j, pt in enumerate(proj(mats["cmat"], "re")):
            nc.scalar.activation(out=re[j], in_=pt, func=AF.Copy)
        im = sbuf2("im")
        for j, pt in enumerate(proj(mats["smat"], "im")):
            nc.scalar.activation(out=im[j], in_=pt, func=AF.Copy)

        # rec = amp * exp(i*pha); amp = |yf|*s1, pha = angle(yf)*s2
        recre, recim = [], []
        for j, (s, l) in enumerate(FC):
            def T(tg):
                return persist.tile([l, BPC], FP32, tag=f"{tg}{j}", name=f"{tg}{j}")

            r2 = T("r2")
            nc.vector.tensor_mul(out=r2, in0=re[j], in1=re[j])
            i2 = T("i2")
            nc.vector.tensor_mul(out=i2, in0=im[j], in1=im[j])
            nc.vector.tensor_add(out=r2, in0=r2, in1=i2)
            amp0 = T("amp0")
            nc.scalar.activation(out=amp0, in_=r2, func=AF.Sqrt)
            den = T("den")  # |z| + Re + eps, then reciprocal
            nc.vector.scalar_tensor_tensor(
                out=den, in0=amp0, scalar=EPS, in1=re[j], op0=OP.add, op1=OP.add
            )
            nc.vector.reciprocal(out=den, in_=den)
            tq = T("tq")  # t = Im / (|z| + Re) = tan(angle/2)
            nc.vector.tensor_mul(out=tq, in0=im[j], in1=den)
            ab = T("ab")  # |t| = max(-t, t), kept strictly positive
            nc.vector.scalar_tensor_tensor(
                out=ab, in0=tq, scalar=-1.0, in1=tq, op0=OP.mult, op1=OP.max
            )
            nc.vector.tensor_scalar_add(out=ab, in0=ab, scalar1=EPS)
            inv = T("inv")
            nc.vector.reciprocal(out=inv, in_=ab)
            u = T("u")
            nc.vector.tensor_tensor(out=u, in0=ab, in1=inv, op=OP.min)
            a = T("a")  # atan(u) in [0, pi/4]
            nc.scalar.activation(out=a, in_=u, func=AF.Arctan)
            w = T("w")  # pi/2 - 2a, applied where |t| > 1
            nc.vector.tensor_scalar(
                out=w, in0=a, scalar1=-2.0, scalar2=float(np.pi / 2),
                op0=OP.mult, op1=OP.add,
            )
            fgt = T("fgt")
            nc.vector.tensor_scalar(
                out=fgt, in0=ab, scalar1=1.0, scalar2=None, op0=OP.is_gt
            )
            nc.vector.tensor_mul(out=w, in0=w, in1=fgt)
            nc.vector.tensor_add(out=a, in0=a, in1=w)  # |angle|/2
            f0 = T("f0")  # sign(t) as 2*(t>0)-1, folded: at = 2*a*f0 - a
            nc.vector.tensor_scalar(
                out=f0, in0=tq, scalar1=0.0, scalar2=None, op0=OP.is_gt
            )
            af = T("af")
            nc.vector.tensor_mul(out=af, in0=a, in1=f0)
            at = T("at")  # angle/2
            nc.vector.scalar_tensor_tensor(
                out=at, in0=af, scalar=2.0, in1=a, op0=OP.mult, op1=OP.subtract
            )
            # DC (j0,p0) / Nyquist (j1,p64) bins: Im==0 analytically, the
            # half-angle quotient is noise-driven. angle is exactly 0 (Re>0)
            # or pi (Re<0): at = pi/2 * (1 - (Re>0)).
            p = 0 if j == 0 else NYQ - FC[1][0]
            fp = persist.tile([1, BPC], FP32, tag=f"fp{j}", name=f"fp{j}")
            nc.vector.tensor_scalar(
                out=fp, in0=re[j][p : p + 1, :], scalar1=0.0, scalar2=None, op0=OP.is_gt
            )
            nc.vector.tensor_scalar(
                out=at[p : p + 1, :], in0=fp,
                scalar1=float(-np.pi / 2), scalar2=float(np.pi / 2),
                op0=OP.mult, op1=OP.add,
            )
            q = T("q")  # q = (angle/2)*s2; pha = 2q
            nc.vector.tensor_mul(out=q, in0=at, in1=s2[j])
            r = T("r")  # pha / 2pi
            nc.vector.tensor_scalar_mul(out=r, in0=q, scalar1=float(1.0 / np.pi))
            rc = T("rc")  # (pha + pi/2) / 2pi
            nc.vector.tensor_scalar_add(out=rc, in0=r, scalar1=0.25)
            n1 = T("n1")
            nc.vector.tensor_scalar(
                out=n1, in0=r, scalar1=MAGIC, scalar2=MAGIC, op0=OP.add, op1=OP.subtract
            )
            nc.vector.tensor_sub(out=n1, in0=r, in1=n1)  # frac(r) in [-.5, .5]
            sn = T("sn")
            nc.scalar.activation(out=sn, in_=n1, func=AF.Sin, scale=float(2 * np.pi))
            n2 = T("n2")
            nc.vector.tensor_scalar(
                out=n2, in0=rc, scalar1=MAGIC, scalar2=MAGIC, op0=OP.add, op1=OP.subtract
            )
            nc.vector.tensor_sub(out=n2, in0=rc, in1=n2)
            cs = T("cs")
            nc.scalar.activation(out=cs, in_=n2, func=AF.Sin, scale=float(2 * np.pi))
            amp = T("amp")
            nc.vector.tensor_mul(out=amp, in0=amp0, in1=s1[j])
            rr = T("rr")
            nc.vector.tensor_mul(out=rr, in0=amp, in1=cs)
            ri = T("ri")
            nc.vector.tensor_mul(out=ri, in0=amp, in1=sn)
            recre.append(rr)
            recim.append(ri)

        # xr = irfft(rec); out = xr*y, PE-transposed to [batch, C] and stored
        # with a single contiguous DMA.
        pfin = ps_fin.tile([BPC, C], FP32, tag="pfin", name="pfin")
        for m, s in enumerate(KC):
            px = ps_mm.tile([128, BPC], FP32, tag="mm", name=f"px{m}")
            steps = [
                (icr_sb[0], recre[0]), (icr_sb[1], recre[1]),
                (ici_sb[0], recim[0]), (ici_sb[1], recim[1]),
            ]
            for idx, (mt, vt) in enumerate(steps):
                nc.tensor.matmul(
                    px, lhsT=mt[:, s : s + 128], rhs=vt,
                    start=(idx == 0), stop=(idx == len(steps) - 1),
                )
            ot = persist.tile([128, BPC], FP32, tag=f"ot{m}", name=f"ot{m}")
            nc.vector.tensor_mul(out=ot, in0=px, in1=y_sb[m])
            nc.tensor.transpose(pfin[:, s : s + 128], ot, id_sb)
        out_sb = persist.tile([BPC, C], FP32, tag="out_sb", name="out_sb")
        nc.vector.tensor_copy(out=out_sb, in_=pfin)
        base = outp.ap()
        dst = bass.AP(tensor=base.tensor, offset=0, ap=[[C, BPC], [1, C]])
        nc.sync.dma_start(out=dst, in_=out_sb)

    nc.compile()
    return nc


_CACHE = {}


def _get_nc():
    if "nc" not in _CACHE:
        _CACHE["nc"] = _build()
    return _CACHE["nc"]


def _host_prep(inputs):
    f32 = np.float32
    W1 = np.asarray(inputs["W1"], f32)
    W2 = np.asarray(inputs["W2"], f32)
    Ws1 = np.asarray(inputs["Ws1"], f32)
    Ws2 = np.asarray(inputs["Ws2"], f32)
    b1 = np.asarray(inputs["b1"], f32)
    b2 = np.asarray(inputs["b2"], f32)
    bs1 = np.asarray(inputs["bs1"], f32)
    bs2 = np.asarray(inputs["bs2"], f32)
    # center taps of the 3x3 convs; fold the 1/HW mean scale into W1
    w1t = (W1[:, :, 1, 1].T.astype(np.float64) / HW).astype(f32)   # [C, CH]
    w2t = np.ascontiguousarray(W2[:, :, 1, 1].T)                   # [CH, C]
    ws1t = np.ascontiguousarray(Ws1.T)                             # [C, CF]
    ws2t = np.ascontiguousarray(Ws2.T)

    i = np.arange(C, dtype=np.float64)[:, None]
    k = np.arange(CF, dtype=np.float64)[None, :]
    ang = 2.0 * np.pi * i * k / C
    cmat = np.cos(ang).astype(f32)                                 # [C, CF]
    smat = (-np.sin(ang)).astype(f32)

    kk = np.arange(CF, dtype=np.float64)[:, None]
    n = np.arange(C, dtype=np.float64)[None, :]
    ang2 = 2.0 * np.pi * kk * n / C
    alpha = np.full((CF, 1), 2.0)
    alpha[0, 0] = 1.0
    alpha[NYQ, 0] = 1.0
    icrm = (alpha * np.cos(ang2) / C).astype(f32)                  # [CF, C]
    icim = (-alpha * np.sin(ang2) / C).astype(f32)

    wpk = np.zeros((128, TOTW), f32)

    def put(name, arr):  # arr: [rows, cols]
        o = _OFF[name]
        wpk[: arr.shape[0], o : o + arr.shape[1]] = arr

    for k3 in range(3):
        put(f"w1t{k3}", w1t[k3 * 128 : (k3 + 1) * 128, :])
    put("w2t", w2t)
    put("b1", b1.reshape(CH, 1))
    for m in range(3):
        put(f"b2_{m}", b2[m * 128 : (m + 1) * 128].reshape(128, 1))
    for nm, mat in (("ws1t", ws1t), ("ws2t", ws2t), ("cmat", cmat), ("smat", smat)):
        for k3 in range(3):
            put(f"{nm}{k3}", mat[k3 * 128 : (k3 + 1) * 128, :])
    for j, (s, l) in enumerate(FC):
        put(f"bs1_{j}", bs1[s : s + l].reshape(l, 1))
        put(f"bs2_{j}", bs2[s : s + l].reshape(l, 1))
        put(f"icr{j}", icrm[s : s + l, :])
        put(f"ici{j}", icim[s : s + l, :])
    put("idmat", np.eye(128, dtype=f32))
    return {"wpk": wpk}


def kernel(**inputs):
    x = np.asarray(inputs["x"], np.float32)
    base = _host_prep(inputs)
    nc = _get_nc()
    in_maps = [
        dict(base, xs=np.ascontiguousarray(x[i * BPC : (i + 1) * BPC]))
        for i in range(NCORES)
    ]
    res = run_bass_kernel_spmd(nc, in_maps, list(range(NCORES))).results
    return np.concatenate([res[i]["out"] for i in range(NCORES)], axis=0)
